# revision 2
# baseline (speedup 1.0000x reference)
"""HMU-layer (omega) Trainium2 kernel.

out[b,n] = exp(-(lam_n*||x_b-mu_n||^2 + sum_k om_nk*((x_b-mu_n)@v_nk)^2)/D)

Strategy (tensor-parallel over n, 8 cores, full I/O):
  Host folds all weight-only terms (fp32):
    vt  = bf16(sqrt(om)*v) laid out (d, n*k)         -> device matmul streams it
    G   = -2*lam*muc - 2*sum_k r*vq   (d, n) bf16    -> one small matmul
    C   = lam*|muc|^2 + sum_k r^2     split hi/lo bf16
  Device per core (n_loc=1024):
    y[b,nk] = xc_bf16 @ vt            (PE, bf16, PSUM f32)
    z = y^2                           (ACT Square / DVE)
    s[b,n] = sum_k z                  (DVE pair-add tree, bf16->f32)
    w[b,n] = xc@G + lam*xc2[b] + C    (PE)
    out = exp(-(s+w)/256)             (ACT Exp)
"""
import sys

sys.path.insert(0, "/opt/trn_rl_repo")

from contextlib import ExitStack

import ml_dtypes
import numpy as np

import concourse.bass as bass
import concourse.tile as tile
from concourse import bacc, mybir
from concourse.bass_utils import run_bass_kernel_spmd
from concourse.masks import make_identity

B, N, D, K = 1024, 8192, 256, 8
NCORES = 8
NLOC = N // NCORES          # 1024 units per core
NKLOC = NLOC * K            # 8192
BT = B // 128               # 8 b-tiles
F32 = mybir.dt.float32
BF16 = mybir.dt.bfloat16
BF = ml_dtypes.bfloat16

# chunk indices (of 8) whose square runs on DVE instead of ACT (load balance)
DVE_SQ = ()


def _kernel_body(tc, out, x, vt, gt, lc):
    nc = tc.nc
    act = mybir.ActivationFunctionType
    with ExitStack() as ctx:
        weights = ctx.enter_context(tc.tile_pool(name="weights", bufs=1))
        xprep = ctx.enter_context(tc.tile_pool(name="xprep", bufs=2))
        zpool = ctx.enter_context(tc.tile_pool(name="zpool", bufs=2))
        spool = ctx.enter_context(tc.tile_pool(name="spool", bufs=2))
        opool = ctx.enter_context(tc.tile_pool(name="opool", bufs=2))
        ypsum = ctx.enter_context(tc.tile_pool(name="ypsum", bufs=2, space="PSUM"))
        wpsum = ctx.enter_context(tc.tile_pool(name="wpsum", bufs=2, space="PSUM"))

        # ---- resident weights ----
        v_tiles = []
        for j in range(8):
            vtile = weights.tile([128, 2, 1024], BF16, tag=f"v{j}")
            for h in range(2):
                nc.sync.dma_start(
                    out=vtile[:, h, :],
                    in_=vt[h * 128 : (h + 1) * 128, j * 1024 : (j + 1) * 1024],
                )
            v_tiles.append(vtile)
        g_sb = weights.tile([128, 2, NLOC], BF16, tag="g")
        for h in range(2):
            nc.sync.dma_start(out=g_sb[:, h, :], in_=gt[h * 128 : (h + 1) * 128, :])
        lc_sb = weights.tile([3, NLOC], BF16, tag="lc")
        nc.sync.dma_start(out=lc_sb, in_=lc)
        ident_bf = weights.tile([128, 128], BF16, tag="idb")
        make_identity(nc, ident_bf)
        ident_f = weights.tile([128, 128], F32, tag="idf")
        make_identity(nc, ident_f)
        # rows: [xc2_bf16; ones; ones]
        xc2ones = weights.tile([3, B], BF16, tag="xc2")
        nc.vector.memset(xc2ones, 1.0)  # rows 1,2 stay ones; row 0 overwritten
        xT = weights.tile([128, 2, B], BF16, tag="xT")

        # ---- x preparation: xc=x-0.5 in bf16, transposed; xc2 row ----
        for i in range(8):
            bs = slice(i * 128, (i + 1) * 128)
            xt = xprep.tile([128, D], F32, tag="xt")
            nc.sync.dma_start(out=xt, in_=x[bs, :])
            xcb = xprep.tile([128, D], BF16, tag="xcb")
            nc.vector.tensor_scalar_add(out=xcb, in0=xt, scalar1=-0.5)
            sqd = xprep.tile([128, D], F32, tag="sqd")
            xc2col = xprep.tile([128, 1], F32, tag="xc2c")
            nc.scalar.activation(
                out=sqd, in_=xcb, func=act.Square, accum_out=xc2col
            )
            for h in range(2):
                tp = ypsum.tile([128, 128], BF16, tag="y")
                nc.tensor.transpose(
                    out=tp, in_=xcb[:, h * 128 : (h + 1) * 128], identity=ident_bf
                )
                nc.scalar.copy(out=xT[:, h, bs], in_=tp)
            tp2 = ypsum.tile([128, 128], F32, tag="y")
            nc.tensor.transpose(out=tp2[0:1, :], in_=xc2col, identity=ident_f)
            nc.scalar.copy(out=xc2ones[0:1, bs], in_=tp2[0:1, :])

        # ---- main loop over b-tiles ----
        for i in range(8):
            bs = slice(i * 128, (i + 1) * 128)
            # w = xc@G + lam*xc2 + C  (PSUM, 2 banks)
            wp = wpsum.tile([128, NLOC], F32, tag="w")
            for nkh in range(2):
                sl = slice(nkh * 512, (nkh + 1) * 512)
                for h in range(2):
                    nc.tensor.matmul(
                        wp[:, sl],
                        lhsT=xT[:, h, bs],
                        rhs=g_sb[:, h, sl],
                        start=(h == 0),
                        stop=False,
                    )
                nc.tensor.matmul(
                    wp[:, sl],
                    lhsT=xc2ones[:, bs],
                    rhs=lc_sb[:, sl],
                    start=False,
                    stop=True,
                )
            # y chunks + squares
            z = zpool.tile([128, NKLOC], BF16, tag="z")
            for j in range(8):
                yp = ypsum.tile([128, 1024], F32, tag="y")
                for half in range(2):
                    ysl = slice(half * 512, (half + 1) * 512)
                    for h in range(2):
                        nc.tensor.matmul(
                            yp[:, ysl],
                            lhsT=xT[:, h, bs],
                            rhs=v_tiles[j][:, h, ysl],
                            start=(h == 0),
                            stop=(h == 1),
                        )
                zj = z[:, j * 1024 : (j + 1) * 1024]
                if j in DVE_SQ:
                    nc.vector.tensor_scalar(
                        out=zj,
                        in0=yp,
                        scalar1=2.0,
                        scalar2=None,
                        op0=mybir.AluOpType.pow,
                    )
                else:
                    nc.scalar.activation(out=zj, in_=yp, func=act.Square)
            # k-reduction tree: 8 -> 4 -> 2 -> 1
            z3 = z.rearrange("p (n k) -> p n k", k=8)
            z4 = spool.tile([128, NLOC * 4], BF16, tag="z4")
            z4r = z4.rearrange("p (n k) -> p n k", k=4)
            nc.vector.tensor_add(out=z4r, in0=z3[:, :, 0:4], in1=z3[:, :, 4:8])
            z2 = spool.tile([128, NLOC * 2], BF16, tag="z2")
            z2r = z2.rearrange("p (n k) -> p n k", k=2)
            nc.vector.tensor_add(out=z2r, in0=z4r[:, :, 0:2], in1=z4r[:, :, 2:4])
            q = spool.tile([128, NLOC], F32, tag="q")
            qr = q.rearrange("p (n k) -> p n k", k=1)
            nc.vector.tensor_add(out=qr, in0=z2r[:, :, 0:1], in1=z2r[:, :, 1:2])
            # merge + exp + store
            q2 = spool.tile([128, NLOC], F32, tag="q2")
            nc.vector.tensor_add(out=q2, in0=q, in1=wp)
            o = opool.tile([128, NLOC], F32, tag="o")
            nc.scalar.activation(out=o, in_=q2, func=act.Exp, scale=-1.0 / D)
            nc.sync.dma_start(out=out[bs, :], in_=o)


_NC_CACHE = None


def _build():
    global _NC_CACHE
    if _NC_CACHE is not None:
        return _NC_CACHE
    nc = bacc.Bacc("TRN2", target_bir_lowering=False, debug=False)
    x_d = nc.dram_tensor("x_in", (B, D), F32, kind="ExternalInput").ap()
    vt_d = nc.dram_tensor("vt_in", (D, NKLOC), BF16, kind="ExternalInput").ap()
    gt_d = nc.dram_tensor("gt_in", (D, NLOC), BF16, kind="ExternalInput").ap()
    lc_d = nc.dram_tensor("lc_in", (3, NLOC), BF16, kind="ExternalInput").ap()
    out_d = nc.dram_tensor("out", (B, NLOC), F32, kind="ExternalOutput").ap()
    with tile.TileContext(nc) as tc:
        _kernel_body(tc, out_d, x_d, vt_d, gt_d, lc_d)
    nc.compile()
    _NC_CACHE = nc
    return nc


def _host_fold(x, mu, lambda_base, v, omega):
    """Weight-only folding + sharding. Returns per-core input maps."""
    x = np.ascontiguousarray(x, dtype=np.float32)
    in_maps = []
    for c in range(NCORES):
        sl = slice(c * NLOC, (c + 1) * NLOC)
        mu_c = mu[sl].astype(np.float32)
        lam_c = lambda_base[sl].astype(np.float32)
        v_c = v[sl].astype(np.float32)
        om_c = omega[sl].astype(np.float32)
        vt = np.sqrt(om_c)[:, :, None] * v_c            # (NLOC, K, D)
        vt_bf = vt.astype(BF)
        vq = vt_bf.astype(np.float32)
        t = 0.5 * vq.sum(-1)                            # (NLOC, K)
        m = np.einsum("nd,nkd->nk", mu_c, vq)
        r = m - t
        muc = mu_c - 0.5
        G = -2.0 * lam_c[:, None] * muc - 2.0 * np.einsum("nk,nkd->nd", r, vq)
        C = lam_c * (muc**2).sum(-1) + (r**2).sum(-1)
        C_hi = C.astype(BF)
        C_lo = (C - C_hi.astype(np.float32)).astype(BF)
        lc_rows = np.stack(
            [lam_c.astype(BF), C_hi, C_lo], axis=0
        )                                               # (3, NLOC) bf16
        # vt layout (D, NLOC*K): vt_t[d, n*K+k] = vt_bf[n, k, d]
        vt_t = np.ascontiguousarray(vt_bf.transpose(2, 0, 1).reshape(D, NKLOC))
        gt = np.ascontiguousarray(G.T.astype(BF))       # (D, NLOC)
        in_maps.append(
            {"x_in": x, "vt_in": vt_t, "gt_in": gt, "lc_in": lc_rows}
        )
    return in_maps


def kernel(x, mu, lambda_base, v, omega, _trace=False, _trace_kwargs=None):
    nc = _build()
    in_maps = _host_fold(x, mu, lambda_base, v, omega)
    res = run_bass_kernel_spmd(
        nc,
        in_maps,
        core_ids=list(range(NCORES)),
        trace=_trace,
        **(_trace_kwargs or {}),
    )
    out = np.concatenate([res.results[c]["out"] for c in range(NCORES)], axis=1)
    if _trace:
        kernel._last_result = res
    return out


# revision 6
# speedup vs baseline: 31.6149x; 31.6149x over previous
"""HMU-layer (omega) Trainium2 kernel.

out[b,n] = exp(-(lam_n*||x_b-mu_n||^2 + sum_k om_nk*((x_b-mu_n)@v_nk)^2)/D)

Strategy (tensor-parallel over n, 8 cores, full I/O):
  Host folds all weight-only terms (fp32):
    vt  = bf16(sqrt(om)*v) laid out (d, n*k)         -> device matmul streams it
    G   = -2*lam*muc - 2*sum_k r*vq   (d, n) bf16    -> one small matmul
    C   = lam*|muc|^2 + sum_k r^2     split hi/lo bf16
  Device per core (n_loc=1024):
    y[b,nk] = xc_bf16 @ vt            (PE, bf16, PSUM f32)
    z = y^2                           (ACT Square / DVE)
    s[b,n] = sum_k z                  (DVE pair-add tree, bf16->f32)
    w[b,n] = xc@G + lam*xc2[b] + C    (PE)
    out = exp(-(s+w)/256)             (ACT Exp)
"""
import sys

sys.path.insert(0, "/opt/trn_rl_repo")

from contextlib import ExitStack

import ml_dtypes
import numpy as np

import concourse.bass as bass
import concourse.tile as tile
from concourse import bacc, mybir
from concourse.bass_utils import run_bass_kernel_spmd
from concourse.masks import make_identity

B, N, D, K = 1024, 8192, 256, 8
NCORES = 8
NLOC = N // NCORES          # 1024 units per core
NKLOC = NLOC * K            # 8192
BT = B // 128               # 8 b-tiles
F32 = mybir.dt.float32
BF16 = mybir.dt.bfloat16
BF = ml_dtypes.bfloat16

# chunk indices (of 8) whose square runs on DVE instead of ACT (load balance)
DVE_SQ = ()


def _kernel_body(tc, out, x, vt, gt, lc, loop_t=1):
    nc = tc.nc
    act = mybir.ActivationFunctionType
    with ExitStack() as ctx:
        weights = ctx.enter_context(tc.tile_pool(name="weights", bufs=1))
        xprep = ctx.enter_context(tc.tile_pool(name="xprep", bufs=2))
        zpool = ctx.enter_context(tc.tile_pool(name="zpool", bufs=2))
        spool = ctx.enter_context(tc.tile_pool(name="spool", bufs=2))
        opool = ctx.enter_context(tc.tile_pool(name="opool", bufs=2))
        ypsum = ctx.enter_context(tc.tile_pool(name="ypsum", bufs=2, space="PSUM"))
        wpsum = ctx.enter_context(tc.tile_pool(name="wpsum", bufs=2, space="PSUM"))

        # ---- resident weights ----
        v_tiles = []
        for j in range(8):
            vtile = weights.tile([128, 2, 1024], BF16, tag=f"v{j}")
            for h in range(2):
                nc.sync.dma_start(
                    out=vtile[:, h, :],
                    in_=vt[h * 128 : (h + 1) * 128, j * 1024 : (j + 1) * 1024],
                )
            v_tiles.append(vtile)
        g_sb = weights.tile([128, 2, NLOC], BF16, tag="g")
        for h in range(2):
            nc.sync.dma_start(out=g_sb[:, h, :], in_=gt[h * 128 : (h + 1) * 128, :])
        lc_sb = weights.tile([3, NLOC], BF16, tag="lc")
        nc.sync.dma_start(out=lc_sb, in_=lc)
        ident_bf = weights.tile([128, 128], BF16, tag="idb")
        make_identity(nc, ident_bf)
        ident_f = weights.tile([128, 128], F32, tag="idf")
        make_identity(nc, ident_f)
        # rows: [xc2_bf16; ones; ones]
        xc2ones = weights.tile([3, B], BF16, tag="xc2")
        nc.vector.memset(xc2ones, 1.0)  # rows 1,2 stay ones; row 0 overwritten
        xT = weights.tile([128, 2, B], BF16, tag="xT")

        # ---- x preparation: xc=x-0.5 in bf16, transposed; xc2 row ----
        for i in range(8):
            bs = slice(i * 128, (i + 1) * 128)
            xt = xprep.tile([128, D], F32, tag="xt")
            nc.sync.dma_start(out=xt, in_=x[bs, :])
            xcb = xprep.tile([128, D], BF16, tag="xcb")
            nc.vector.tensor_scalar_add(out=xcb, in0=xt, scalar1=-0.5)
            sqd = xprep.tile([128, D], F32, tag="sqd")
            xc2col = xprep.tile([128, 1], F32, tag="xc2c")
            nc.scalar.activation(
                out=sqd, in_=xcb, func=act.Square, accum_out=xc2col
            )
            for h in range(2):
                tp = ypsum.tile([128, 128], BF16, tag="y")
                nc.tensor.transpose(
                    out=tp, in_=xcb[:, h * 128 : (h + 1) * 128], identity=ident_bf
                )
                nc.scalar.copy(out=xT[:, h, bs], in_=tp)
            tp2 = ypsum.tile([128, 128], F32, tag="y")
            nc.tensor.transpose(out=tp2[0:1, :], in_=xc2col, identity=ident_f)
            nc.scalar.copy(out=xc2ones[0:1, bs], in_=tp2[0:1, :])

        # ---- main loop over b-tiles ----
        if loop_t > 1:
            loop_cm = tc.For_i(0, loop_t, 1, hint_engines=(mybir.EngineType.PE,))
            loop_cm.__enter__()
        for i in range(8):
            bs = slice(i * 128, (i + 1) * 128)
            # w = xc@G + lam*xc2 + C  (PSUM, 2 banks)
            wp = wpsum.tile([128, NLOC], F32, tag="w")
            for nkh in range(2):
                sl = slice(nkh * 512, (nkh + 1) * 512)
                for h in range(2):
                    nc.tensor.matmul(
                        wp[:, sl],
                        lhsT=xT[:, h, bs],
                        rhs=g_sb[:, h, sl],
                        start=(h == 0),
                        stop=False,
                    )
                nc.tensor.matmul(
                    wp[:, sl],
                    lhsT=xc2ones[:, bs],
                    rhs=lc_sb[:, sl],
                    start=False,
                    stop=True,
                )
            # y chunks + squares
            z = zpool.tile([128, NKLOC], BF16, tag="z")
            for j in range(8):
                yp = ypsum.tile([128, 1024], F32, tag="y")
                for half in range(2):
                    ysl = slice(half * 512, (half + 1) * 512)
                    for h in range(2):
                        nc.tensor.matmul(
                            yp[:, ysl],
                            lhsT=xT[:, h, bs],
                            rhs=v_tiles[j][:, h, ysl],
                            start=(h == 0),
                            stop=(h == 1),
                        )
                zj = z[:, j * 1024 : (j + 1) * 1024]
                if j in DVE_SQ:
                    nc.vector.tensor_scalar(
                        out=zj,
                        in0=yp,
                        scalar1=2.0,
                        scalar2=None,
                        op0=mybir.AluOpType.pow,
                    )
                else:
                    nc.scalar.activation(out=zj, in_=yp, func=act.Square)
            # k-reduction tree: 8 -> 4 -> 2 -> 1
            z3 = z.rearrange("p (n k) -> p n k", k=8)
            z4 = spool.tile([128, NLOC * 4], BF16, tag="z4")
            z4r = z4.rearrange("p (n k) -> p n k", k=4)
            nc.vector.tensor_add(out=z4r, in0=z3[:, :, 0:4], in1=z3[:, :, 4:8])
            z2 = spool.tile([128, NLOC * 2], BF16, tag="z2")
            z2r = z2.rearrange("p (n k) -> p n k", k=2)
            nc.vector.tensor_add(out=z2r, in0=z4r[:, :, 0:2], in1=z4r[:, :, 2:4])
            q = spool.tile([128, NLOC], F32, tag="q")
            qr = q.rearrange("p (n k) -> p n k", k=1)
            nc.vector.tensor_add(out=qr, in0=z2r[:, :, 0:1], in1=z2r[:, :, 1:2])
            # merge + exp + store
            q2 = spool.tile([128, NLOC], F32, tag="q2")
            nc.vector.tensor_add(out=q2, in0=q, in1=wp)
            o = opool.tile([128, NLOC], F32, tag="o")
            nc.scalar.activation(out=o, in_=q2, func=act.Exp, scale=-1.0 / D)
            nc.sync.dma_start(out=out[bs, :], in_=o)
        if loop_t > 1:
            loop_cm.__exit__(None, None, None)


_NC_CACHE = {}


def _build(loop_t=1):
    if loop_t in _NC_CACHE:
        return _NC_CACHE[loop_t]
    nc = bacc.Bacc("TRN2", target_bir_lowering=False, debug=False)
    x_d = nc.dram_tensor("x_in", (B, D), F32, kind="ExternalInput").ap()
    vt_d = nc.dram_tensor("vt_in", (D, NKLOC), BF16, kind="ExternalInput").ap()
    gt_d = nc.dram_tensor("gt_in", (D, NLOC), BF16, kind="ExternalInput").ap()
    lc_d = nc.dram_tensor("lc_in", (3, NLOC), BF16, kind="ExternalInput").ap()
    out_d = nc.dram_tensor("out", (B, NLOC), F32, kind="ExternalOutput").ap()
    with tile.TileContext(nc) as tc:
        _kernel_body(tc, out_d, x_d, vt_d, gt_d, lc_d, loop_t=loop_t)
    nc.compile()
    _NC_CACHE[loop_t] = nc
    return nc


def _host_fold(x, mu, lambda_base, v, omega):
    """Weight-only folding + sharding. Returns per-core input maps."""
    x = np.ascontiguousarray(x, dtype=np.float32)
    in_maps = []
    for c in range(NCORES):
        sl = slice(c * NLOC, (c + 1) * NLOC)
        mu_c = mu[sl].astype(np.float32)
        lam_c = lambda_base[sl].astype(np.float32)
        v_c = v[sl].astype(np.float32)
        om_c = omega[sl].astype(np.float32)
        vt = np.sqrt(om_c)[:, :, None] * v_c            # (NLOC, K, D)
        vt_bf = vt.astype(BF)
        vq = vt_bf.astype(np.float32)
        t = 0.5 * vq.sum(-1)                            # (NLOC, K)
        m = np.einsum("nd,nkd->nk", mu_c, vq)
        r = m - t
        muc = mu_c - 0.5
        G = -2.0 * lam_c[:, None] * muc - 2.0 * np.einsum("nk,nkd->nd", r, vq)
        C = lam_c * (muc**2).sum(-1) + (r**2).sum(-1)
        C_hi = C.astype(BF)
        C_lo = (C - C_hi.astype(np.float32)).astype(BF)
        lc_rows = np.stack(
            [lam_c.astype(BF), C_hi, C_lo], axis=0
        )                                               # (3, NLOC) bf16
        # vt layout (D, NLOC*K): vt_t[d, n*K+k] = vt_bf[n, k, d]
        vt_t = np.ascontiguousarray(vt_bf.transpose(2, 0, 1).reshape(D, NKLOC))
        gt = np.ascontiguousarray(G.T.astype(BF))       # (D, NLOC)
        in_maps.append(
            {"x_in": x, "vt_in": vt_t, "gt_in": gt, "lc_in": lc_rows}
        )
    return in_maps


def kernel(x, mu, lambda_base, v, omega, _trace=False, _trace_kwargs=None):
    nc = _build()
    in_maps = _host_fold(x, mu, lambda_base, v, omega)
    res = run_bass_kernel_spmd(
        nc,
        in_maps,
        core_ids=list(range(NCORES)),
        trace=_trace,
        **(_trace_kwargs or {}),
    )
    out = np.concatenate([res.results[c]["out"] for c in range(NCORES)], axis=1)
    if _trace:
        kernel._last_result = res
    return out
